# revision 1
# baseline (speedup 1.0000x reference)
"""Bass/Tile TRN2 kernel for BasicAttention.

att = softmax(tanh(hidden @ W_h.T + p_att_feats) @ W_alpha + mask) @ att_feats

Shapes: B=64, N=2048, H=1024, A=512. Data-parallel over batch across 8
NeuronCores (8 batches per core); weights replicated; no collectives.

Layout: region index n maps to (partition p, column c) as n = p*16 + c so
every p_att/att_feats DMA is a long contiguous per-partition read and the
mask tile is a natural row-major reshape.

Per-core dataflow (memory-bound: ~96MB HBM reads/core at ~358 GB/s):
  host: pass W_h.T, hidden.T and a pre-broadcast bf16 W_alpha (layout-only
        transforms) so no PE transposes are needed on device.
  setup: w_h = hidden @ W_h.T (PE) -> per-batch partition-broadcast of
         w_h rows via a DRAM round-trip 0-stride DMA.
  per batch b (software-pipelined, p_att phase leads att_feats phase):
    p_att stream [128,8,512]: DVE add (w_h bcast) -> ACT tanh (bf16)
      -> DVE scalar_tensor_tensor vs W_alpha (accum) -> scores[128,16]
    scores: + mask, ACT exp (accum rowsum, f32r out), PE total-sum,
      DVE reciprocal
    att_feats stream [128,4,1024] f32r: PE matmuls (attn col stationary)
      accumulating att[1,1024] in PSUM -> DVE scale by 1/sum -> out.
"""

import numpy as np

B, N, H, A = 64, 2048, 1024, 512
NCORES = 8
BLOC = B // NCORES  # batches per core

P = 128
NT = N // P            # 16 n-columns per partition
PATT_SUP = 8           # columns per p_att supertile (2 DMAs per batch)
AF_SUP = 4             # columns per att_feats supertile (4 DMAs per batch)

_NC_CACHE = {}


def _free_bcast(bass_mod, ap, repeat):
    """[P, F] AP -> [P, repeat, F] AP with 0-stride middle dim."""
    return bass_mod.AP(
        tensor=ap.tensor,
        offset=ap.offset,
        ap=[ap.ap[0], [0, repeat], *ap.ap[1:]],
    )


def _build_nc():
    import concourse.bass as bass
    import concourse.mybir as mybir
    import concourse.tile as tile
    from concourse import bacc

    dt = mybir.dt
    f32, f32r, bf16 = dt.float32, dt.float32r, dt.bfloat16
    AF = mybir.ActivationFunctionType
    OP = mybir.AluOpType

    nc = bacc.Bacc("TRN2", target_bir_lowering=False, debug=False,
                   num_devices=NCORES)

    hsT = nc.dram_tensor("hidden_T", [H, BLOC], f32, kind="ExternalInput").ap()
    af = nc.dram_tensor("att_feats", [BLOC, N, H], f32r, kind="ExternalInput").ap()
    pa = nc.dram_tensor("p_att_feats", [BLOC, N, A], f32, kind="ExternalInput").ap()
    am = nc.dram_tensor("att_masks", [BLOC, N], f32, kind="ExternalInput").ap()
    whT = nc.dram_tensor("W_hT", [H, A], f32, kind="ExternalInput").ap()
    wab = nc.dram_tensor("W_alpha_b", [P, A], bf16, kind="ExternalInput").ap()
    out = nc.dram_tensor("att_out", [BLOC, H], f32, kind="ExternalOutput").ap()

    with tile.TileContext(nc) as tc:
        with (
            tc.tile_pool(name="consts", bufs=1) as consts,
            tc.tile_pool(name="patt", bufs=4) as patt_pool,
            tc.tile_pool(name="alpha", bufs=3) as alpha_pool,
            tc.tile_pool(name="afp", bufs=3) as af_pool,
            tc.tile_pool(name="small", bufs=4) as small,
            tc.tile_pool(name="psmisc", bufs=2, space="PSUM") as psmisc,
            tc.tile_pool(name="psatt", bufs=6, space="PSUM") as psatt,
        ):
            # ---------------- setup ----------------
            ones_col = consts.tile([P, 1], f32)
            nc.vector.memset(ones_col, 1.0)

            whT_sb = []
            for hc in range(H // P):  # 8 tiles [128h, 512a], contiguous rows
                t = consts.tile([P, A], f32, name=f"whT{hc}", tag=f"whT{hc}")
                nc.sync.dma_start(out=t, in_=whT[hc * P:(hc + 1) * P, :])
                whT_sb.append(t)
            hidT_sb = []
            for hc in range(H // P):  # 8 tiles [128h, 8b]
                t = consts.tile([P, BLOC], f32, name=f"hidT{hc}", tag=f"hidT{hc}")
                nc.sync.dma_start(out=t, in_=hsT[hc * P:(hc + 1) * P, :])
                hidT_sb.append(t)
            wa_bf = consts.tile([P, A], bf16)
            nc.sync.dma_start(out=wa_bf, in_=wab[:, :])

            # w_h = hidden @ W_h.T : [8, 512]
            wh_ps = psmisc.tile([BLOC, A], f32, tag="mm")
            for hc in range(H // P):
                nc.tensor.matmul(wh_ps, lhsT=hidT_sb[hc], rhs=whT_sb[hc],
                                 start=(hc == 0), stop=(hc == H // P - 1))
            whall_sb = consts.tile([BLOC, A], f32)
            nc.vector.tensor_copy(whall_sb, wh_ps)

            # per-batch w_h row broadcast to [128, 512] f32 via a DRAM
            # round-trip with a 0-stride partition AP (setup-only, ~2MB)
            whall_dram = nc.dram_tensor("whall_scratch", [BLOC, A], f32).ap()
            nc.sync.dma_start(out=whall_dram, in_=whall_sb)
            whb = []
            for b in range(BLOC):
                t = consts.tile([P, A], f32, name=f"whb{b}", tag=f"whb{b}")
                row = whall_dram[b:b + 1, :]
                src = bass.AP(tensor=row.tensor, offset=row.offset,
                              ap=[[0, P], row.ap[1]])
                nc.sync.dma_start(out=t, in_=src)
                whb.append(t)

            # ---------------- main loop (software-pipelined) ----------------
            # n = p*NT + c everywhere below.
            pa_r = [pa[b, :, :].rearrange("(p c) a -> p c a", c=NT)
                    for b in range(BLOC)]
            af_r = [af[b, :, :].rearrange("(p c) h -> p c h", c=NT)
                    for b in range(BLOC)]

            def patt_phase(b):
                scores = small.tile([P, NT], f32, tag="scores",
                                    name=f"scores{b}")
                for st in range(NT // PATT_SUP):  # 2 supertiles
                    pt = patt_pool.tile([P, PATT_SUP, A], f32, tag="patt",
                                        name=f"patt{b}_{st}")
                    nc.sync.dma_start(
                        out=pt,
                        in_=pa_r[b][:, st * PATT_SUP:(st + 1) * PATT_SUP, :],
                    )
                    whb_b = _free_bcast(bass, whb[b][:, :], PATT_SUP)
                    nc.vector.tensor_tensor(out=pt, in0=pt, in1=whb_b, op=OP.add)
                    ab = alpha_pool.tile([P, PATT_SUP, A], bf16, tag="alpha",
                                         name=f"alpha{b}_{st}")
                    nc.scalar.activation(ab, pt, AF.Tanh)
                    for c in range(PATT_SUP):
                        col = st * PATT_SUP + c
                        # out = (ab * 1) * wa ; accum_out = row-sum -> scores
                        nc.vector.scalar_tensor_tensor(
                            out=ab[:, c, :], in0=ab[:, c, :], scalar=1.0,
                            in1=wa_bf, op0=OP.mult, op1=OP.mult,
                            accum_out=scores[:, col:col + 1],
                        )

                masks = small.tile([P, NT], f32, tag="masks", name=f"masks{b}")
                nc.sync.dma_start(
                    out=masks, in_=am[b, :].rearrange("(p c) -> p c", c=NT))
                nc.vector.tensor_tensor(out=scores, in0=scores, in1=masks,
                                        op=OP.add)

                expt = small.tile([P, NT], f32r, tag="expt", name=f"expt{b}")
                rowsum = small.tile([P, 1], f32, tag="rowsum", name=f"rowsum{b}")
                nc.scalar.activation(expt, scores, AF.Exp, accum_out=rowsum)

                sum_ps = psmisc.tile([1, 1], f32, tag="mm", name=f"sum_ps{b}")
                nc.tensor.matmul(sum_ps, lhsT=rowsum, rhs=ones_col,
                                 start=True, stop=True)
                inv = small.tile([1, 1], f32, tag="inv", name=f"inv{b}")
                nc.vector.reciprocal(inv, sum_ps)
                return expt, inv

            def af_phase(b, expt, inv):
                att_lo = psatt.tile([1, A], f32, tag="att", name=f"attlo{b}")
                att_hi = psatt.tile([1, A], f32, tag="att", name=f"atthi{b}")
                for st2 in range(NT // AF_SUP):
                    aft = af_pool.tile([P, AF_SUP, H], f32r, tag="af",
                                       name=f"af{b}_{st2}")
                    nc.sync.dma_start(
                        out=aft,
                        in_=af_r[b][:, st2 * AF_SUP:(st2 + 1) * AF_SUP, :],
                    )
                    for c in range(AF_SUP):
                        t = st2 * AF_SUP + c
                        lhs = expt[:, t:t + 1]
                        nc.tensor.matmul(att_lo, lhsT=lhs,
                                         rhs=aft[:, c, 0:A],
                                         start=(t == 0), stop=(t == NT - 1))
                        nc.tensor.matmul(att_hi, lhsT=lhs,
                                         rhs=aft[:, c, A:H],
                                         start=(t == 0), stop=(t == NT - 1))

                att_row = small.tile([1, H], f32, tag="attrow",
                                     name=f"attrow{b}")
                nc.vector.tensor_scalar_mul(att_row[:, 0:A], att_lo, inv)
                nc.vector.tensor_scalar_mul(att_row[:, A:H], att_hi, inv)
                nc.sync.dma_start(out=out[b:b + 1, :], in_=att_row)

            state = {}
            for b in range(BLOC):
                state[b] = patt_phase(b)
                if b >= 1:
                    af_phase(b - 1, *state.pop(b - 1))
            af_phase(BLOC - 1, *state.pop(BLOC - 1))

    nc.compile()
    return nc


def _get_nc():
    if "nc" not in _NC_CACHE:
        _NC_CACHE["nc"] = _build_nc()
    return _NC_CACHE["nc"]


def kernel(hidden_states, att_feats, p_att_feats, att_masks, W_h, W_alpha):
    import ml_dtypes
    from concourse.bass_utils import run_bass_kernel_spmd

    nc = _get_nc()
    hidden_states = np.ascontiguousarray(hidden_states, dtype=np.float32)
    att_feats = np.ascontiguousarray(att_feats, dtype=np.float32)
    p_att_feats = np.ascontiguousarray(p_att_feats, dtype=np.float32)
    att_masks = np.ascontiguousarray(att_masks, dtype=np.float32)
    W_h = np.ascontiguousarray(W_h, dtype=np.float32)
    W_alpha = np.asarray(W_alpha, dtype=np.float32).reshape(1, A)

    whT = np.ascontiguousarray(W_h.T)                       # [H, A]
    wab = np.ascontiguousarray(
        np.broadcast_to(W_alpha, (P, A))).astype(ml_dtypes.bfloat16)

    in_maps = []
    for i in range(NCORES):
        s = slice(i * BLOC, (i + 1) * BLOC)
        in_maps.append({
            "hidden_T": np.ascontiguousarray(hidden_states[s].T),
            "att_feats": att_feats[s],
            "p_att_feats": p_att_feats[s],
            "att_masks": att_masks[s],
            "W_hT": whT,
            "W_alpha_b": wab,
        })

    global _LAST_IN_MAPS
    _LAST_IN_MAPS = in_maps
    res = run_bass_kernel_spmd(nc, in_maps, core_ids=list(range(NCORES)))
    return np.concatenate(
        [res.results[i]["att_out"] for i in range(NCORES)], axis=0
    ).astype(np.float32)


_LAST_IN_MAPS = None



# revision 3
# speedup vs baseline: 1.8144x; 1.8144x over previous
"""Bass/Tile TRN2 kernel for BasicAttention (v2: bf16 inputs + PE scores).

att = softmax(tanh(hidden @ W_h.T + p_att_feats) @ W_alpha + mask) @ att_feats

Shapes: B=64, N=2048, H=1024, A=512. Data-parallel over batch across 8
NeuronCores (8 batches per core); weights replicated; no collectives.

v2 strategy (memory-bound; HBM floor halves with bf16 inputs):
  host: cast att_feats -> bf16 [B,N,H]; cast+transpose p_att_feats ->
        bf16 [B,A,N]; bf16 hidden_T/W_hT; W_alpha as [128,4] bf16.
  device per core:
    setup: wh_T[a,b] = (W_h @ hidden.T) computed directly transposed on PE
           (lhsT=W_hT tiles, rhs=hidden_T) -> 4 bias tiles [128a, 8b] f32.
    per batch b (pipelined):
      pa_T stream [128a, 4ab, 2048n] bf16: ACT tanh with per-partition
        bias wh_T[:, b] fused (no DVE add) -> alpha_T bf16.
      scores on PE: for c in 16: lhsT = alpha_T[:, ab, c::16] (stationary,
        M=128 regions), rhs = W_alpha chunk [128a, 1] -> sps[128p, c]
        accumulating over 4 ablocks; lands directly in n = p*16+c layout.
      mask add (DVE), exp+rowsum (ACT, bf16 out), total sum (PE ones),
        reciprocal (DVE).
      att_feats stream [128p, 8c, 1024h] bf16: 32 PE matmuls
        (attn col stationary [128,1]) -> att [1,1024] PSUM f32,
        scale by 1/sum (DVE), store.
Engine budget/core: DMA ~137us (bound), PE ~100us, ACT ~67us, DVE ~25us.
"""

import numpy as np

B, N, H, A = 64, 2048, 1024, 512
NCORES = 8
BLOC = B // NCORES  # batches per core

P = 128
NT = N // P       # 16 n-columns per partition (n = p*16 + c)
AB = A // P       # 4 a-blocks
HC = H // P       # 8 h-blocks
AF_SUP = 8        # att_feats columns per supertile (2 DMAs per batch)

_NC_CACHE = {}


def _build_nc():
    import concourse.bass as bass
    import concourse.mybir as mybir
    import concourse.tile as tile
    from concourse import bacc

    dt = mybir.dt
    f32, bf16 = dt.float32, dt.bfloat16
    AF = mybir.ActivationFunctionType
    OP = mybir.AluOpType

    nc = bacc.Bacc("TRN2", target_bir_lowering=False, debug=False,
                   num_devices=NCORES)

    paT = nc.dram_tensor("p_att_T", [BLOC, A, N], bf16, kind="ExternalInput").ap()
    af = nc.dram_tensor("att_feats", [BLOC, N, H], bf16, kind="ExternalInput").ap()
    am = nc.dram_tensor("att_masks", [BLOC, N], f32, kind="ExternalInput").ap()
    hsT = nc.dram_tensor("hidden_T", [H, BLOC], bf16, kind="ExternalInput").ap()
    whT = nc.dram_tensor("W_hT", [H, A], bf16, kind="ExternalInput").ap()
    wa4 = nc.dram_tensor("W_alpha4", [P, AB], bf16, kind="ExternalInput").ap()
    out = nc.dram_tensor("att_out", [BLOC, H], f32, kind="ExternalOutput").ap()

    with tile.TileContext(nc) as tc:
        with (
            tc.tile_pool(name="consts", bufs=1) as consts,
            tc.tile_pool(name="patt", bufs=2) as pa_pool,
            tc.tile_pool(name="alpha", bufs=2) as alpha_pool,
            tc.tile_pool(name="afp", bufs=5) as af_pool,
            tc.tile_pool(name="small", bufs=4) as small,
            tc.tile_pool(name="psmisc", bufs=2, space="PSUM") as psmisc,
            tc.tile_pool(name="psscore", bufs=2, space="PSUM") as psscore,
            tc.tile_pool(name="psatt", bufs=4, space="PSUM") as psatt,
        ):
            # ---------------- setup ----------------
            ones_col = consts.tile([P, 1], f32, tag="ones")
            nc.vector.memset(ones_col, 1.0)

            # W_hT as [128h, 8hc, 512a] (one DMA)
            whT_sb = consts.tile([P, HC, A], bf16, tag="whT")
            nc.sync.dma_start(
                out=whT_sb, in_=whT.rearrange("(hc p) a -> p hc a", p=P))
            # hidden_T as [128h, 8hc, 8b] (one DMA)
            hidT_sb = consts.tile([P, HC, BLOC], bf16, tag="hidT")
            nc.sync.dma_start(
                out=hidT_sb, in_=hsT.rearrange("(hc p) b -> p hc b", p=P))
            # W_alpha as [128, 4ab]
            wa_sb = consts.tile([P, AB], bf16, tag="wa")
            nc.sync.dma_start(out=wa_sb, in_=wa4)
            # all masks [128p, 8b, 16c] (one DMA)
            masks_sb = consts.tile([P, BLOC, NT], f32, tag="masks")
            nc.sync.dma_start(
                out=masks_sb, in_=am.rearrange("b (p c) -> p b c", p=P))

            # wh_T = W_h @ hidden.T computed transposed: [128a, 8b] x 4
            wh_bias = []
            for ac in range(AB):
                ps = psmisc.tile([P, BLOC], f32, tag="mm", name=f"whps{ac}")
                for hc in range(HC):
                    nc.tensor.matmul(
                        ps,
                        lhsT=whT_sb[:, hc, ac * P:(ac + 1) * P],
                        rhs=hidT_sb[:, hc, :],
                        start=(hc == 0), stop=(hc == HC - 1))
                t = consts.tile([P, BLOC], f32, tag=f"whb{ac}")
                nc.vector.tensor_copy(t, ps)
                wh_bias.append(t)

            # ---------------- main loop (software-pipelined) ----------------
            paT_r = [paT[b, :, :].rearrange("(ab p) n -> p ab n", p=P)
                     for b in range(BLOC)]
            af_r = [af[b, :, :].rearrange("(p c) h -> p c h", c=NT)
                    for b in range(BLOC)]

            af_tiles = {}

            def patt_phase(b):
                pa_t = pa_pool.tile([P, AB, N], bf16, tag="pa", name=f"pa{b}")
                nc.sync.dma_start(out=pa_t, in_=paT_r[b])
                # prefetch att_feats for this batch
                tiles = []
                for st in range(NT // AF_SUP):
                    aft = af_pool.tile([P, AF_SUP, H], bf16, tag="af",
                                       name=f"af{b}_{st}")
                    nc.sync.dma_start(
                        out=aft,
                        in_=af_r[b][:, st * AF_SUP:(st + 1) * AF_SUP, :])
                    tiles.append(aft)
                af_tiles[b] = tiles

                alpha_t = alpha_pool.tile([P, AB, N], bf16, tag="alpha",
                                          name=f"alpha{b}")
                for ab in range(AB):
                    nc.scalar.activation(
                        alpha_t[:, ab, :], pa_t[:, ab, :], AF.Tanh,
                        bias=wh_bias[ab][:, b:b + 1])
                return alpha_t

            def scores_phase(b, alpha_t):
                sps = psscore.tile([P, NT], f32, tag="sps", name=f"sps{b}")
                for c in range(NT):
                    for ab in range(AB):
                        # stationary = alpha_T[:, ab, c::16]  (128 n's with
                        # stride 16 -> M-dim partition p of the output)
                        nc.tensor.matmul(
                            sps[:, c:c + 1], lhsT=alpha_t[:, ab, c::NT],
                            rhs=wa_sb[:, ab:ab + 1],
                            start=(ab == 0), stop=(ab == AB - 1))

                scores = small.tile([P, NT], f32, tag="scores",
                                    name=f"scores{b}")
                nc.vector.tensor_tensor(out=scores, in0=sps,
                                        in1=masks_sb[:, b, :], op=OP.add)
                expt = small.tile([P, NT], bf16, tag="expt", name=f"expt{b}")
                rowsum = small.tile([P, 1], f32, tag="rowsum",
                                    name=f"rowsum{b}")
                nc.scalar.activation(expt, scores, AF.Exp, accum_out=rowsum)

                sum_ps = psmisc.tile([1, 1], f32, tag="mm", name=f"sum_ps{b}")
                nc.tensor.matmul(sum_ps, lhsT=rowsum, rhs=ones_col,
                                 start=True, stop=True)
                inv = small.tile([1, 1], f32, tag="inv", name=f"inv{b}")
                nc.vector.reciprocal(inv, sum_ps)
                return expt, inv

            def af_phase(b, expt, inv):
                att_lo = psatt.tile([1, A], f32, tag="att", name=f"attlo{b}")
                att_hi = psatt.tile([1, A], f32, tag="att", name=f"atthi{b}")
                for st in range(NT // AF_SUP):
                    aft = af_tiles[b][st]
                    for c in range(AF_SUP):
                        t = st * AF_SUP + c
                        lhs = expt[:, t:t + 1]
                        nc.tensor.matmul(att_lo, lhsT=lhs,
                                         rhs=aft[:, c, 0:A],
                                         start=(t == 0), stop=(t == NT - 1))
                        nc.tensor.matmul(att_hi, lhsT=lhs,
                                         rhs=aft[:, c, A:H],
                                         start=(t == 0), stop=(t == NT - 1))
                del af_tiles[b]

                att_row = small.tile([1, H], f32, tag="attrow",
                                     name=f"attrow{b}")
                nc.vector.tensor_scalar_mul(att_row[:, 0:A], att_lo, inv)
                nc.vector.tensor_scalar_mul(att_row[:, A:H], att_hi, inv)
                nc.sync.dma_start(out=out[b:b + 1, :], in_=att_row)

            state = {}
            for b in range(BLOC):
                alpha_t = patt_phase(b)
                if b >= 1:
                    af_phase(b - 1, *state.pop(b - 1))
                state[b] = scores_phase(b, alpha_t)
            af_phase(BLOC - 1, *state.pop(BLOC - 1))

    nc.compile()
    return nc


def _get_nc():
    if "nc" not in _NC_CACHE:
        _NC_CACHE["nc"] = _build_nc()
    return _NC_CACHE["nc"]


def kernel(hidden_states, att_feats, p_att_feats, att_masks, W_h, W_alpha):
    import ml_dtypes
    from concourse.bass_utils import run_bass_kernel_spmd

    nc = _get_nc()
    bf16 = ml_dtypes.bfloat16

    af16 = np.ascontiguousarray(att_feats).astype(bf16)           # [B,N,H]
    paT16 = np.ascontiguousarray(
        np.ascontiguousarray(p_att_feats).astype(bf16).transpose(0, 2, 1))
    am32 = np.ascontiguousarray(att_masks, dtype=np.float32)      # [B,N]
    hsT16 = np.ascontiguousarray(hidden_states, dtype=np.float32).T.astype(bf16)
    whT16 = np.ascontiguousarray(
        np.ascontiguousarray(W_h, dtype=np.float32).T).astype(bf16)  # [H,A]
    wa16 = np.ascontiguousarray(
        np.asarray(W_alpha, dtype=np.float32).reshape(AB, P).T).astype(bf16)

    in_maps = []
    for i in range(NCORES):
        s = slice(i * BLOC, (i + 1) * BLOC)
        in_maps.append({
            "p_att_T": paT16[s],
            "att_feats": af16[s],
            "att_masks": am32[s],
            "hidden_T": np.ascontiguousarray(hsT16[:, s]),
            "W_hT": whT16,
            "W_alpha4": wa16,
        })

    global _LAST_IN_MAPS
    _LAST_IN_MAPS = in_maps
    res = run_bass_kernel_spmd(nc, in_maps, core_ids=list(range(NCORES)))
    return np.concatenate(
        [res.results[i]["att_out"] for i in range(NCORES)], axis=0
    ).astype(np.float32)


_LAST_IN_MAPS = None


# revision 6
# speedup vs baseline: 1.9658x; 1.0834x over previous
"""Bass/Tile TRN2 kernel for BasicAttention (v2: bf16 inputs + PE scores).

att = softmax(tanh(hidden @ W_h.T + p_att_feats) @ W_alpha + mask) @ att_feats

Shapes: B=64, N=2048, H=1024, A=512. Data-parallel over batch across 8
NeuronCores (8 batches per core); weights replicated; no collectives.

v2 strategy (memory-bound; HBM floor halves with bf16 inputs):
  host: cast att_feats -> bf16 [B,N,H]; cast+transpose p_att_feats ->
        bf16 [B,A,N]; bf16 hidden_T/W_hT; W_alpha as [128,4] bf16.
  device per core:
    setup: wh_T[a,b] = (W_h @ hidden.T) computed directly transposed on PE
           (lhsT=W_hT tiles, rhs=hidden_T) -> 4 bias tiles [128a, 8b] f32.
    per batch b (pipelined):
      pa_T stream [128a, 4ab, 2048n] bf16: ACT tanh with per-partition
        bias wh_T[:, b] fused (no DVE add) -> alpha_T bf16.
      scores on PE: for c in 16: lhsT = alpha_T[:, ab, c::16] (stationary,
        M=128 regions), rhs = W_alpha chunk [128a, 1] -> sps[128p, c]
        accumulating over 4 ablocks; lands directly in n = p*16+c layout.
      mask add (DVE), exp+rowsum (ACT, bf16 out), total sum (PE ones),
        reciprocal (DVE).
      att_feats stream [128p, 8c, 1024h] bf16: 32 PE matmuls
        (attn col stationary [128,1]) -> att [1,1024] PSUM f32,
        scale by 1/sum (DVE), store.
Engine budget/core: DMA ~137us (bound), PE ~100us, ACT ~67us, DVE ~25us.
"""

import numpy as np

B, N, H, A = 64, 2048, 1024, 512
NCORES = 8
BLOC = B // NCORES  # batches per core

P = 128
NT = N // P       # 16 n-columns per partition (n = p*16 + c)
AB = A // P       # 4 a-blocks
HC = H // P       # 8 h-blocks
AF_SUP = 8        # att_feats columns per supertile (2 DMAs per batch)

_NC_CACHE = {}


def _build_nc():
    import concourse.bass as bass
    import concourse.mybir as mybir
    import concourse.tile as tile
    from concourse import bacc

    dt = mybir.dt
    f32, bf16 = dt.float32, dt.bfloat16
    AF = mybir.ActivationFunctionType
    OP = mybir.AluOpType

    nc = bacc.Bacc("TRN2", target_bir_lowering=False, debug=False,
                   num_devices=NCORES)

    paT = nc.dram_tensor("p_att_T", [BLOC, A, N], bf16, kind="ExternalInput").ap()
    af = nc.dram_tensor("att_feats", [BLOC, N, H], bf16, kind="ExternalInput").ap()
    am = nc.dram_tensor("att_masks", [BLOC, N], f32, kind="ExternalInput").ap()
    hsT = nc.dram_tensor("hidden_T", [H, BLOC], bf16, kind="ExternalInput").ap()
    whT = nc.dram_tensor("W_hT", [H, A], bf16, kind="ExternalInput").ap()
    wa4 = nc.dram_tensor("W_alpha4", [P, AB], bf16, kind="ExternalInput").ap()
    out = nc.dram_tensor("att_out", [BLOC, H], f32, kind="ExternalOutput").ap()

    with tile.TileContext(nc) as tc:
        with (
            tc.tile_pool(name="consts", bufs=1) as consts,
            tc.tile_pool(name="patt", bufs=3) as pa_pool,
            tc.tile_pool(name="alpha", bufs=2) as alpha_pool,
            tc.tile_pool(name="afp", bufs=6) as af_pool,
            tc.tile_pool(name="small", bufs=6) as small,
            tc.tile_pool(name="arow", bufs=2) as arow_pool,
            tc.tile_pool(name="psmisc", bufs=2, space="PSUM") as psmisc,
            tc.tile_pool(name="psscore", bufs=2, space="PSUM") as psscore,
            tc.tile_pool(name="psatt", bufs=4, space="PSUM") as psatt,
        ):
            # ---------------- setup ----------------
            ones_col = consts.tile([P, 1], f32, tag="ones")
            nc.vector.memset(ones_col, 1.0)

            # W_hT as [128h, 8hc, 512a] (one DMA)
            whT_sb = consts.tile([P, HC, A], bf16, tag="whT")
            nc.sync.dma_start(
                out=whT_sb, in_=whT.rearrange("(hc p) a -> p hc a", p=P))
            # hidden_T as [128h, 8hc, 8b] (one DMA)
            hidT_sb = consts.tile([P, HC, BLOC], bf16, tag="hidT")
            nc.sync.dma_start(
                out=hidT_sb, in_=hsT.rearrange("(hc p) b -> p hc b", p=P))
            # W_alpha as [128, 4ab]
            wa_sb = consts.tile([P, AB], bf16, tag="wa")
            nc.sync.dma_start(out=wa_sb, in_=wa4)
            # all masks [128p, 8b, 16c] (one DMA)
            masks_sb = consts.tile([P, BLOC, NT], f32, tag="masks")
            nc.sync.dma_start(
                out=masks_sb, in_=am.rearrange("b (p c) -> p b c", p=P))

            # wh_T = W_h @ hidden.T computed transposed: [128a, 8b] x 4
            wh_bias = []
            for ac in range(AB):
                ps = psmisc.tile([P, BLOC], f32, tag="mm", name=f"whps{ac}")
                for hc in range(HC):
                    nc.tensor.matmul(
                        ps,
                        lhsT=whT_sb[:, hc, ac * P:(ac + 1) * P],
                        rhs=hidT_sb[:, hc, :],
                        start=(hc == 0), stop=(hc == HC - 1))
                t = consts.tile([P, BLOC], f32, tag=f"whb{ac}")
                nc.vector.tensor_copy(t, ps)
                wh_bias.append(t)

            # ---------------- main loop (software-pipelined) ----------------
            paT_r = [paT[b, :, :].rearrange("(ab p) n -> p ab n", p=P)
                     for b in range(BLOC)]
            af_r = [af[b, :, :].rearrange("(p c) h -> p c h", c=NT)
                    for b in range(BLOC)]

            af_tiles = {}

            def patt_phase(b):
                pa_t = pa_pool.tile([P, AB, N], bf16, tag="pa", name=f"pa{b}")
                nc.sync.dma_start(out=pa_t, in_=paT_r[b])
                # prefetch att_feats for this batch
                tiles = []
                for st in range(NT // AF_SUP):
                    aft = af_pool.tile([P, AF_SUP, H], bf16, tag="af",
                                       name=f"af{b}_{st}")
                    nc.sync.dma_start(
                        out=aft,
                        in_=af_r[b][:, st * AF_SUP:(st + 1) * AF_SUP, :])
                    tiles.append(aft)
                af_tiles[b] = tiles

                alpha_t = alpha_pool.tile([P, AB, N], bf16, tag="alpha",
                                          name=f"alpha{b}")
                for ab in range(AB):
                    nc.scalar.activation(
                        alpha_t[:, ab, :], pa_t[:, ab, :], AF.Tanh,
                        bias=wh_bias[ab][:, b:b + 1])
                return alpha_t

            def scores_phase(b, alpha_t):
                sps = psscore.tile([P, NT], f32, tag="sps", name=f"sps{b}")
                for c in range(NT):
                    for ab in range(AB):
                        # stationary = alpha_T[:, ab, c::16]  (128 n's with
                        # stride 16 -> M-dim partition p of the output)
                        nc.tensor.matmul(
                            sps[:, c:c + 1], lhsT=alpha_t[:, ab, c::NT],
                            rhs=wa_sb[:, ab:ab + 1],
                            start=(ab == 0), stop=(ab == AB - 1))

                scores = small.tile([P, NT], f32, tag="scores",
                                    name=f"scores{b}")
                nc.vector.tensor_tensor(out=scores, in0=sps,
                                        in1=masks_sb[:, b, :], op=OP.add)
                expt = small.tile([P, NT], bf16, tag="expt", name=f"expt{b}")
                rowsum = small.tile([P, 1], f32, tag="rowsum",
                                    name=f"rowsum{b}")
                nc.scalar.activation(expt, scores, AF.Exp, accum_out=rowsum)

                sum_ps = psmisc.tile([1, 1], f32, tag="mm", name=f"sum_ps{b}")
                nc.tensor.matmul(sum_ps, lhsT=rowsum, rhs=ones_col,
                                 start=True, stop=True)
                inv = small.tile([1, 1], f32, tag="inv", name=f"inv{b}")
                nc.vector.reciprocal(inv, sum_ps)
                return expt, inv

            def af_phase(b, expt, inv):
                att_lo = psatt.tile([1, A], f32, tag="att", name=f"attlo{b}")
                att_hi = psatt.tile([1, A], f32, tag="att", name=f"atthi{b}")
                for st in range(NT // AF_SUP):
                    aft = af_tiles[b][st]
                    for c in range(AF_SUP):
                        t = st * AF_SUP + c
                        lhs = expt[:, t:t + 1]
                        nc.tensor.matmul(att_lo, lhsT=lhs,
                                         rhs=aft[:, c, 0:A],
                                         start=(t == 0), stop=(t == NT - 1))
                        nc.tensor.matmul(att_hi, lhsT=lhs,
                                         rhs=aft[:, c, A:H],
                                         start=(t == 0), stop=(t == NT - 1))
                del af_tiles[b]

                att_row = arow_pool.tile([1, H], f32, tag="attrow",
                                         name=f"attrow{b}")
                nc.vector.tensor_scalar_mul(att_row[:, 0:A], att_lo, inv)
                nc.vector.tensor_scalar_mul(att_row[:, A:H], att_hi, inv)
                nc.sync.dma_start(out=out[b:b + 1, :], in_=att_row)

            # Schedule: att phases run with a 2-batch skew (so the PE never
            # waits on the tanh->scores->exp chain) and are PAIRED into long
            # high-duty bursts so the PE clock-gate (HAM) warms once per pair
            # instead of once per batch: att batches run at iters
            # 2:(0,) 3:(1,2) 5:(3,4) 7:(5,6) end:(7,).
            att_sched = {2: (0,), 3: (1, 2), 5: (3, 4), 7: (5, 6)}
            state = {}
            for b in range(BLOC):
                alpha_t = patt_phase(b)
                for ab_ in att_sched.get(b, ()):
                    af_phase(ab_, *state.pop(ab_))
                state[b] = scores_phase(b, alpha_t)
            af_phase(BLOC - 1, *state.pop(BLOC - 1))

    nc.compile()
    return nc


def _get_nc():
    if "nc" not in _NC_CACHE:
        _NC_CACHE["nc"] = _build_nc()
    return _NC_CACHE["nc"]


def kernel(hidden_states, att_feats, p_att_feats, att_masks, W_h, W_alpha):
    import ml_dtypes
    from concourse.bass_utils import run_bass_kernel_spmd

    nc = _get_nc()
    bf16 = ml_dtypes.bfloat16

    af16 = np.ascontiguousarray(att_feats).astype(bf16)           # [B,N,H]
    paT16 = np.ascontiguousarray(
        np.ascontiguousarray(p_att_feats).astype(bf16).transpose(0, 2, 1))
    am32 = np.ascontiguousarray(att_masks, dtype=np.float32)      # [B,N]
    hsT16 = np.ascontiguousarray(hidden_states, dtype=np.float32).T.astype(bf16)
    whT16 = np.ascontiguousarray(
        np.ascontiguousarray(W_h, dtype=np.float32).T).astype(bf16)  # [H,A]
    wa16 = np.ascontiguousarray(
        np.asarray(W_alpha, dtype=np.float32).reshape(AB, P).T).astype(bf16)

    in_maps = []
    for i in range(NCORES):
        s = slice(i * BLOC, (i + 1) * BLOC)
        in_maps.append({
            "p_att_T": paT16[s],
            "att_feats": af16[s],
            "att_masks": am32[s],
            "hidden_T": np.ascontiguousarray(hsT16[:, s]),
            "W_hT": whT16,
            "W_alpha4": wa16,
        })

    global _LAST_IN_MAPS
    _LAST_IN_MAPS = in_maps
    res = run_bass_kernel_spmd(nc, in_maps, core_ids=list(range(NCORES)))
    return np.concatenate(
        [res.results[i]["att_out"] for i in range(NCORES)], axis=0
    ).astype(np.float32)


_LAST_IN_MAPS = None
